# revision 12
# baseline (speedup 1.0000x reference)
"""Collective variant: K/V projection split across core pairs + pair AllGather.

Core c = (batch c//2, stripe h = c%2). Each core projects K^T and V only for
its own key half (s in [h*1024, (h+1)*1024)), then the pair exchanges halves
via two AllGathers (one per 512-key own-block) so attention can start as soon
as the first halves have been gathered.

Gathered DRAM layout (per 512-key global block b, r = b//2 = producing rank,
sub = b%2 selects which of the two collectives):
  cc = ccA if b%2==0 else ccB; base = r*2048
  KT tile k:  cc[base + k*128 : +128, :]                       [128, 512]
  V tile st:  cc[base + 1024 + st*256 : +256, :] as [128,1024] (row-pair fold)
"""

import numpy as np

B, S, E, KD = 4, 2048, 1024, 1024
NCORES = 8
P = 128
ET = E // P
KT = KD // P
NQT = 8
NBLK = 4
NEG = -30000.0
SCALE = 1.0 / float(np.sqrt(KD))

_prog_cache = {}


def _n_blocks(t):
    return (t + 2) // 2


def _build_body(ctx, tc, ap):
    from concourse import mybir
    from concourse.masks import make_identity

    nc = tc.nc
    f32 = mybir.dt.float32
    f32r = mybir.dt.float32r
    Exp = mybir.ActivationFunctionType.Exp
    X = mybir.AxisListType.X

    xTq_t = ap["xTq"].rearrange("(t p) q -> t p q", p=P)    # [8, 128, 1024]
    xTp_t = ap["xTp"].rearrange("(t p) s -> t p s", p=P)    # [8, 128, 1024]
    wqT_t = ap["wqT"].rearrange("(t p) k -> t p k", p=P)
    wkT_t = ap["wkT"].rearrange("(t p) k -> t p k", p=P)
    wvT_t = ap["wvT"].rearrange("(t p) f -> t p f", p=P)
    out_t = ap["out"].rearrange("(t p) f -> t p f", p=P)

    # ---- persistent tiles
    qt_pool = ctx.enter_context(tc.tile_pool(name="qt", bufs=1))
    QT = [qt_pool.tile([P, 1024], f32r, name=f"qt{k}", tag=f"qt{k}") for k in range(KT)]
    acc_pool = ctx.enter_context(tc.tile_pool(name="acc", bufs=1))
    OACC = [acc_pool.tile([P, E], f32, name=f"oacc{t}", tag=f"oacc{t}") for t in range(NQT)]
    RS = [acc_pool.tile([P, NBLK], f32, name=f"rs{t}", tag=f"rs{t}") for t in range(NQT)]
    const_pool = ctx.enter_context(tc.tile_pool(name="const", bufs=1))
    cm = const_pool.tile([P, 256], f32, name="cm")
    nc.sync.dma_start(out=cm, in_=ap["cmask"])
    ident_f32 = const_pool.tile([P, P], f32, name="ident_f32")
    make_identity(nc, ident_f32)
    ident = const_pool.tile([P, P], f32r, name="ident")
    nc.vector.tensor_copy(ident, ident_f32)
    fin_pool = ctx.enter_context(tc.tile_pool(name="fin", bufs=3))

    # ---- DRAM tiles for the pair exchange
    dram = ctx.enter_context(tc.tile_pool(name="dram", bufs=1, space="DRAM"))
    ccin = [dram.tile([2048, 512], f32r, name=f"ccin{i}", tag=f"ccin{i}") for i in range(2)]
    ccout = [dram.tile([4096, 512], f32r, name=f"ccout{i}", tag=f"ccout{i}") for i in range(2)]

    # ---- PSUM pools (8 banks)
    pp = ctx.enter_context(tc.tile_pool(name="pp", bufs=2, space="PSUM"))
    sp = ctx.enter_context(tc.tile_pool(name="sp", bufs=2, space="PSUM"))
    tp = ctx.enter_context(tc.tile_pool(name="tp", bufs=2, space="PSUM"))
    vp = ctx.enter_context(tc.tile_pool(name="vp", bufs=1, space="PSUM"))

    # ---- Phase A: QT[k, q] projection
    with tc.tile_pool(name="wqp", bufs=1) as wq_pool, \
         tc.tile_pool(name="xqp", bufs=1) as xq_pool:
        wq = [wq_pool.tile([P, KD], f32r, name=f"wq{e}", tag=f"wq{e}") for e in range(ET)]
        xq = [xq_pool.tile([P, 1024], f32r, name=f"xq{e}", tag=f"xq{e}") for e in range(ET)]
        for e in range(ET):
            nc.sync.dma_start(out=wq[e], in_=wqT_t[e])
            nc.sync.dma_start(out=xq[e], in_=xTq_t[e])
        for qb in range(2):
            for k in range(KT):
                ps = pp.tile([P, 512], f32, name="ps_qt", tag="pp")
                for e in range(ET):
                    nc.tensor.matmul(
                        ps, wq[e][:, k * P:(k + 1) * P],
                        xq[e][:, qb * 512:(qb + 1) * 512],
                        start=(e == 0), stop=(e == ET - 1))
                nc.vector.tensor_copy(QT[k][:, qb * 512:(qb + 1) * 512], ps)

    # ---- Phase B: own-half K/V projection + pair exchange
    with tc.tile_pool(name="wkp", bufs=1) as wk_pool, \
         tc.tile_pool(name="wvp", bufs=1) as wv_pool, \
         tc.tile_pool(name="xpp", bufs=1) as xp_pool, \
         tc.tile_pool(name="stg", bufs=2) as stg_pool:
        wk = [wk_pool.tile([P, KD], f32r, name=f"wk{e}", tag=f"wk{e}") for e in range(ET)]
        xp = [xp_pool.tile([P, 1024], f32r, name=f"xp{e}", tag=f"xp{e}") for e in range(ET)]
        wv = [wv_pool.tile([P, E], f32r, name=f"wv{e}", tag=f"wv{e}") for e in range(ET)]
        for e in range(ET):
            nc.sync.dma_start(out=xp[e], in_=xTp_t[e])
            nc.sync.dma_start(out=wk[e], in_=wkT_t[e])
        for e in range(ET):
            nc.sync.dma_start(out=wv[e], in_=wvT_t[e])

        for ob in range(2):
            # KT_own[ob]: [kd=1024, 512]
            for k in range(KT):
                ps = pp.tile([P, 512], f32, name="ps_kt", tag="pp")
                for e in range(ET):
                    nc.tensor.matmul(ps, wk[e][:, k * P:(k + 1) * P],
                                     xp[e][:, ob * 512:(ob + 1) * 512],
                                     start=(e == 0), stop=(e == ET - 1))
                ko = stg_pool.tile([P, 512], f32r, name="ko", tag="ko")
                nc.vector.tensor_copy(ko, ps)
                nc.sync.dma_start(out=ccin[ob][k * P:(k + 1) * P, :], in_=ko)
            # V_own[ob]: [512, 1024] -> rows 1024: as [1024, 512] row-pair fold
            for st in range(4):
                vo = stg_pool.tile([P, E], f32r, name="vo", tag="vo")
                for fb in range(2):
                    ps = pp.tile([P, 512], f32, name="ps_v", tag="pp")
                    for e in range(ET):
                        nc.tensor.matmul(
                            ps, xp[e][:, ob * 512 + st * P: ob * 512 + (st + 1) * P],
                            wv[e][:, fb * 512:(fb + 1) * 512],
                            start=(e == 0), stop=(e == ET - 1))
                    nc.scalar.copy(vo[:, fb * 512:(fb + 1) * 512], ps)
                vdst = ccin[ob][1024 + st * 256: 1024 + (st + 1) * 256, :]
                nc.sync.dma_start(
                    out=vdst.rearrange("(s a) c -> s (a c)", a=2), in_=vo)
            nc.gpsimd.collective_compute(
                "AllGather", mybir.AluOpType.bypass,
                replica_groups=[[0, 1], [2, 3], [4, 5], [6, 7]],
                ins=[ccin[ob].opt()], outs=[ccout[ob].opt()],
            )

    # ---- Phase C: attention over global blocks
    kt_pool = ctx.enter_context(tc.tile_pool(name="ktp", bufs=2))
    vb_pool = ctx.enter_context(tc.tile_pool(name="vbp", bufs=2))
    p_pool = ctx.enter_context(tc.tile_pool(name="ppb", bufs=3))
    pt_pool = ctx.enter_context(tc.tile_pool(name="ptp", bufs=4))

    ORDER = (0, 2, 1, 3)  # blocks 0,2 come from CC1 — start before CC2 lands
    last_visit = {t: [b for b in ORDER if t >= 2 * b][-1] for t in range(NQT)}
    for blk in ORDER:
        r, sub = blk // 2, blk % 2
        cc = ccout[sub]
        base = r * 2048
        ktb = [kt_pool.tile([P, 512], f32r, name=f"ktb{k}", tag=f"ktb{k}") for k in range(KT)]
        for k in range(KT):
            nc.sync.dma_start(out=ktb[k], in_=cc[base + k * P: base + (k + 1) * P, :])
        vbt = [vb_pool.tile([P, E], f32r, name=f"vb{st}", tag=f"vb{st}") for st in range(4)]
        for st in range(4):
            vsrc = cc[base + 1024 + st * 256: base + 1024 + (st + 1) * 256, :]
            nc.sync.dma_start(out=vbt[st], in_=vsrc.rearrange("(s a) c -> s (a c)", a=2))

        for t in range(2 * blk, NQT):
            w = min(512, 256 * (t + 1) - 512 * blk)
            is_diag = (blk == _n_blocks(t) - 1)   # causal boundary block
            is_last = (blk == last_visit[t])      # last visit in ORDER
            sps = sp.tile([P, 512], f32, name="sps", tag="sp")
            for k in range(KT):
                nc.tensor.matmul(sps[:, :w], QT[k][:, t * P:(t + 1) * P],
                                 ktb[k][:, :w], start=(k == 0), stop=(k == KT - 1))
            if is_diag:
                nc.vector.tensor_add(sps[:, w - 256:w], sps[:, w - 256:w], cm)
            pb = p_pool.tile([P, 512], f32r, name="pb", tag="pb")
            nc.scalar.activation(pb[:, :w], sps[:, :w], Exp, scale=SCALE,
                                 accum_out=RS[t][:, blk:blk + 1])
            nst = w // P
            vps = [vp.tile([P, 512], f32, name=f"vps{fb}", tag=f"vp{fb}") for fb in range(2)]
            for st in range(nst):
                tps = tp.tile([P, P], f32r, name="tps", tag="tp")
                nc.tensor.transpose(tps, pb[:, st * P:(st + 1) * P], ident)
                pts = pt_pool.tile([P, P], f32r, name="pts", tag="pt")
                nc.vector.tensor_copy(pts, tps)
                for fb in range(2):
                    nc.tensor.matmul(vps[fb], pts,
                                     vbt[st][:, fb * 512:(fb + 1) * 512],
                                     start=(st == 0), stop=(st == nst - 1))
            for fb in range(2):
                dst = OACC[t][:, fb * 512:(fb + 1) * 512]
                if blk == 0:
                    nc.vector.tensor_copy(dst, vps[fb])
                else:
                    nc.vector.tensor_add(dst, dst, vps[fb])

            if is_last:
                nb = _n_blocks(t)
                rsum = fin_pool.tile([P, 1], f32, name="rsum", tag="rsum")
                nc.vector.reduce_sum(rsum, RS[t][:, :nb], axis=X)
                rinv = fin_pool.tile([P, 1], f32, name="rinv", tag="rinv")
                nc.vector.reciprocal(rinv, rsum)
                nc.vector.tensor_scalar_mul(OACC[t], OACC[t], rinv)
                nc.sync.dma_start(out=out_t[t], in_=OACC[t])


def build_program():
    if "nc" in _prog_cache:
        return _prog_cache["nc"]
    from contextlib import ExitStack
    from concourse import bacc, mybir
    import concourse.tile as tile

    nc = bacc.Bacc("TRN2", target_bir_lowering=False, debug=False,
                   num_devices=NCORES)
    f32 = mybir.dt.float32
    f32r = mybir.dt.float32r
    ap = {
        "xTq": nc.dram_tensor("xTq", [E, 1024], f32r, kind="ExternalInput").ap(),
        "xTp": nc.dram_tensor("xTp", [E, 1024], f32r, kind="ExternalInput").ap(),
        "wqT": nc.dram_tensor("wqT", [E, KD], f32r, kind="ExternalInput").ap(),
        "wkT": nc.dram_tensor("wkT", [E, KD], f32r, kind="ExternalInput").ap(),
        "wvT": nc.dram_tensor("wvT", [E, E], f32r, kind="ExternalInput").ap(),
        "cmask": nc.dram_tensor("cmask", [P, 256], f32, kind="ExternalInput").ap(),
        "out": nc.dram_tensor("out", [1024, E], f32, kind="ExternalOutput").ap(),
    }
    with tile.TileContext(nc) as tc:
        with ExitStack() as ctx:
            _build_body(ctx, tc, ap)
    nc.compile()
    _prog_cache["nc"] = nc
    return nc


def make_in_maps(x, W_q, W_k, W_v):
    x = np.ascontiguousarray(np.asarray(x, np.float32))
    wqT = np.ascontiguousarray(np.asarray(W_q, np.float32).T)
    wkT = np.ascontiguousarray(np.asarray(W_k, np.float32).T)
    wvT = np.ascontiguousarray(np.asarray(W_v, np.float32).T)
    i = np.arange(P)[:, None]
    j = np.arange(256)[None, :]
    cmasks = [np.where(j <= i + 128, 0.0, NEG).astype(np.float32),
              np.where(j <= i, 0.0, NEG).astype(np.float32)]
    in_maps = []
    for c in range(NCORES):
        b, h = c // 2, c % 2
        xT = np.ascontiguousarray(x[b].T)
        qtiles = [2 * t + (1 - h) for t in range(NQT)]
        qcols = np.concatenate([np.arange(g * P, (g + 1) * P) for g in qtiles])
        xTq = np.ascontiguousarray(xT[:, qcols])
        xTp = np.ascontiguousarray(xT[:, h * 1024:(h + 1) * 1024])
        in_maps.append({
            "xTq": xTq, "xTp": xTp, "wqT": wqT, "wkT": wkT, "wvT": wvT,
            "cmask": cmasks[h],
        })
    return in_maps


def assemble(results):
    out = np.zeros((B, S, E), np.float32)
    for c in range(NCORES):
        b, h = c // 2, c % 2
        co = results[c]["out"]
        for t in range(NQT):
            g = 2 * t + (1 - h)
            out[b, g * P:(g + 1) * P, :] = co[t * P:(t + 1) * P]
    return out


def kernel(x, W_q, W_k, W_v):
    from concourse.bass_utils import run_bass_kernel_spmd
    nc = build_program()
    in_maps = make_in_maps(x, W_q, W_k, W_v)
    res = run_bass_kernel_spmd(nc, in_maps, core_ids=list(range(NCORES)))
    return assemble(res.results)
